# revision 7
# baseline (speedup 1.0000x reference)
"""Trainium2 Bass kernel for nn_ContrastiveLoss (B=4096, D=1024, 8 cores).

loss = mean over [B,B] of
    labels*(1-sim0) + (1-labels)*relu(sim0-0.5)
  + labels*(1-sim1) + (1-labels)*relu(sim1-0.5)
where sim_k = cos_sim(fc_feats_k[i], textual_features[j]).

Strategy (data-parallel over rows):
  * Each of the 8 cores gets a 512-row slice of fc_feats_0/1 and labels.
  * textual_features row-shard is normalized+transposed to fp8 and
    AllGathered so every core holds the full tn^T.
  * f0/f1 are NOT normalized: raw rows are cast to fp8 and transposed;
    1/||f_i|| is applied as a per-partition ACT scale in the relu pass
    and factored out of the bilinear sums algebraically.
  * Per S-tile [128,1024] (2 j-chunks) in PSUM:
      ACT:  r = relu(rin_f * S - 0.5), accum -> racc   (A term + r data)
      DVE:  accum L * S_raw            -> pacc         (x rin_f later)
      DVE:  w = r0 + r1 (bf16, 2x rate)
      Pool: accum (w - 2) * L          -> qacc         (folds C and 2*sum(L))
  * total_core = sum_i [ A - q - rin0*p0 - rin1*p1 ]; host sums cores / B^2.

Self-contained: hardcodes shapes; only needs the concourse package.
"""

import os
import sys

import numpy as np

B = 4096
D = 1024
NCORES = 8
ROWS = B // NCORES          # 512 rows of f0/f1/labels per core
IT = ROWS // 128            # 4 i-tiles per core
KS = D // 128               # 8 k-subtiles (contraction)
JC = B // 512               # 8 j-chunks of 512 columns
JQ = 4                      # j "quarter" = 2 j-chunks = 1024 cols
MARGIN = 0.5
EPS = 1e-8
TN_SCALE = 64.0             # fp8 scale on normalized t rows

_CACHE = {}

# 2 = split the tn^T AllGather into two k-half collectives (pipelining);
# 1 = single collective.
N_COLL = int(os.environ.get("KERNEL_N_COLL", "1"))


def _import_concourse():
    try:
        import concourse.bass  # noqa: F401
    except ImportError:
        for p in ("/opt/trn_rl_repo", "/root/.axon_site/_ro/trn_rl_repo"):
            if os.path.isdir(p) and p not in sys.path:
                sys.path.insert(0, p)
        import concourse.bass  # noqa: F401


def _build_nc():
    """Build + schedule + compile the per-core Bass program (SPMD: same
    program on all 8 cores, different input slices)."""
    _import_concourse()
    import concourse.bass as bass  # noqa: F401
    import concourse.mybir as mybir
    import concourse.tile as tile
    from concourse import bacc
    from concourse.masks import make_identity

    f32 = mybir.dt.float32
    bf16 = mybir.dt.bfloat16
    fp8 = mybir.dt.float8e4
    AF = mybir.ActivationFunctionType
    OP = mybir.AluOpType
    AX = mybir.AxisListType
    DR = mybir.MatmulPerfMode.DoubleRow

    nc = bacc.Bacc(
        "TRN2",
        target_bir_lowering=False,
        debug=False,
        num_devices=NCORES,
    )

    f0_d = nc.dram_tensor("f0", [ROWS, D], f32, kind="ExternalInput").ap()
    f1_d = nc.dram_tensor("f1", [ROWS, D], f32, kind="ExternalInput").ap()
    tx_d = nc.dram_tensor("tx", [ROWS, D], f32, kind="ExternalInput").ap()
    lab_d = nc.dram_tensor("lab", [ROWS, B], f32, kind="ExternalInput").ap()
    out_d = nc.dram_tensor("outv", [128, 1], f32, kind="ExternalOutput").ap()

    with tile.TileContext(nc) as tc:
        with (
            tc.tile_pool(name="constp", bufs=1) as constp,
            tc.tile_pool(name="stage", bufs=4) as stage,
            tc.tile_pool(name="natbp", bufs=3) as natbp,
            tc.tile_pool(name="sqp", bufs=2) as sqp,
            tc.tile_pool(name="small", bufs=8) as small,
            tc.tile_pool(name="wT", bufs=1) as wTp,
            tc.tile_pool(name="tnTp", bufs=1) as tnTp,
            tc.tile_pool(name="labp", bufs=4) as labp,
            tc.tile_pool(name="rbufp", bufs=4) as rbufp,
            tc.tile_pool(name="scrp", bufs=4) as scrp,
            tc.tile_pool(name="accp", bufs=1) as accp,
            tc.tile_pool(name="tpsum", bufs=2, space="PSUM") as tpsum,
            tc.tile_pool(name="mpsum", bufs=3, space="PSUM") as mpsum,
            tc.tile_pool(name="dram", bufs=1, space="DRAM") as dram,
        ):
            ident = constp.tile([128, 128], bf16)
            make_identity(nc, ident)
            negmargin = constp.tile([128, 1], f32)
            nc.gpsimd.memset(negmargin, -MARGIN)

            # persistent per-(ic,f) inverse norms (col 2*ic+f), scaled 1/64
            rinp = accp.tile([128, 2 * IT], f32)
            # accumulators, each column written exactly once
            racc = accp.tile([128, 2 * IT * JQ], f32)   # sum relu, (ic,jq,f)
            pacc = accp.tile([128, 2 * IT * JQ], f32)   # sum L*Sraw, (ic,f,jq)
            qacc = accp.tile([128, IT * JQ], f32)       # sum (w-2)*L, (ic,jq)

            # evacuation-copy engine rotation
            _evac_cnt = [0]

            def evac(dst, src):
                # Pool cannot read PSUM; alternate DVE / ACT only.
                e = _evac_cnt[0] % 2
                _evac_cnt[0] += 1
                if e == 0:
                    nc.vector.tensor_copy(dst, src)
                else:
                    nc.scalar.copy(dst, src)

            def norm_chain(nat, tag, scale_mul):
                """Square+accum -> sqrt -> clamp -> reciprocal -> *scale_mul.
                Returns the [128,1] f32 inverse-norm tile (not yet stored)."""
                sqb = sqp.tile([128, D], bf16, tag="sqb", name=f"sqb_{tag}")
                ssq = small.tile([128, 1], f32, tag="ssq", name=f"ssq_{tag}")
                nc.scalar.activation(sqb, nat, AF.Square, accum_out=ssq)
                nrm = small.tile([128, 1], f32, tag="nrm", name=f"nrm_{tag}")
                nc.scalar.activation(nrm, ssq, AF.Sqrt)
                nc.vector.tensor_scalar_max(nrm, nrm, EPS)
                rin = small.tile([128, 1], f32, tag="rin", name=f"rin_{tag}")
                nc.vector.reciprocal(rin, nrm)
                if scale_mul != 1.0:
                    nc.vector.tensor_scalar_mul(rin, rin, scale_mul)
                return rin

            def transpose_into(src_b, dst_T, it):
                """PE-transpose [128, D] bf16 tile into dst_T[:, ks, it*128:...]
                (fp8, layout [d_part, ks, row])."""
                for ks in range(KS):
                    pst = tpsum.tile([128, 128], bf16, tag="pst",
                                     name=f"pst_{dst_T.tensor.name}_{it}_{ks}")
                    nc.tensor.transpose(pst, src_b[:, ks * 128:(ks + 1) * 128],
                                        ident)
                    evac(dst_T[:, ks, it * 128:(it + 1) * 128], pst)

            # ---- phase A1: tx slice -> normalize -> transpose -> gather ----
            tT_loc = wTp.tile([128, KS, ROWS], fp8)
            for it in range(IT):
                nat = stage.tile([128, D], f32, tag="nat", name=f"tnat_{it}")
                nc.sync.dma_start(nat, tx_d[it * 128:(it + 1) * 128, :])
                rin = norm_chain(nat, f"t{it}", TN_SCALE)
                tnb = natbp.tile([128, D], bf16, tag="natb", name=f"tnb_{it}")
                nc.vector.tensor_scalar_mul(tnb, nat, rin)
                transpose_into(tnb, tT_loc, it)

            # share tn^T via AllGather (1 or 2 collectives of k-halves)
            kh = KS // N_COLL
            tT_loc_ds, tT_all_ds = [], []
            for h in range(N_COLL):
                tld = dram.tile([kh, 128, ROWS], fp8, name=f"tT_loc_d{h}")
                nc.gpsimd.dma_start(
                    tld.rearrange("ks p j -> p ks j"),
                    tT_loc[:, h * kh:(h + 1) * kh, :])
                tad = dram.tile([NCORES, kh, 128, ROWS], fp8,
                                addr_space="Shared", name=f"tT_all_d{h}")
                nc.gpsimd.collective_compute(
                    "AllGather",
                    mybir.AluOpType.bypass,
                    replica_groups=[list(range(NCORES))],
                    ins=[tld.opt()],
                    outs=[tad.opt()],
                )
                tT_loc_ds.append(tld)
                tT_all_ds.append(tad)

            # ---- phase A2: f0/f1 raw -> cast -> norms -> transpose ----
            f0T = wTp.tile([128, KS, ROWS], fp8)
            f1T = wTp.tile([128, KS, ROWS], fp8)
            for f, (src_ap, fT) in enumerate(((f0_d, f0T), (f1_d, f1T))):
                for it in range(IT):
                    nat = stage.tile([128, D], f32, tag="nat",
                                     name=f"fnat_{f}_{it}")
                    nc.sync.dma_start(nat, src_ap[it * 128:(it + 1) * 128, :])
                    natb = natbp.tile([128, D], bf16, tag="natb",
                                      name=f"fnatb_{f}_{it}")
                    nc.gpsimd.tensor_copy(natb, nat)
                    rin = norm_chain(nat, f"f{f}_{it}", 1.0 / TN_SCALE)
                    nc.vector.tensor_copy(rinp[:, 2 * it + f:2 * it + f + 1],
                                          rin)
                    transpose_into(natb, fT, it)

            # ---- labels: fully resident, loaded early; bf16 copy for the
            # all-bf16 (4x DVE) t3 pass ----
            Lts, Lbs = [], []
            for ic in range(IT):
                Lt = labp.tile([128, B], f32, tag="Lt", name=f"Lt_{ic}")
                nc.sync.dma_start(Lt, lab_d[ic * 128:(ic + 1) * 128, :])
                Lb = labp.tile([128, B], bf16, tag="Lb", name=f"Lb_{ic}")
                nc.gpsimd.tensor_copy(Lb, Lt)
                Lts.append(Lt)
                Lbs.append(Lb)

            # ---- gathered tn^T -> SBUF [128, jc*KS + ks, 512] ----
            tnT = tnTp.tile([128, JC * KS, 512], fp8)
            for jc in range(JC):
                for h in range(N_COLL):
                    nc.sync.dma_start(
                        tnT[:, jc * KS + h * kh: jc * KS + (h + 1) * kh, :],
                        tT_all_ds[h][jc].rearrange("ks p j -> p ks j"),
                    )

            # ---- phase B: matmuls + fused loss ----
            for ic in range(IT):
                isl = slice(ic * 128, (ic + 1) * 128)
                Lt = Lts[ic]
                for jq in range(JQ):
                    jsl = slice(jq * 1024, (jq + 1) * 1024)
                    rbufs = []
                    for f, fT in enumerate((f0T, f1T)):
                        ps = mpsum.tile([128, 1024], f32, tag="ps",
                                        name=f"ps_{ic}_{jq}_{f}")
                        for jh in range(2):
                            jc = jq * 2 + jh
                            for k2 in range(KS // 2):
                                ksl = slice(jc * KS + 2 * k2,
                                            jc * KS + 2 * k2 + 2)
                                nc.tensor.matmul(
                                    ps[:, jh * 512:(jh + 1) * 512],
                                    fT[:, 2 * k2:2 * k2 + 2, isl],
                                    tnT[:, ksl, :], perf_mode=DR,
                                    start=(k2 == 0), stop=(k2 == KS // 2 - 1),
                                )
                        rsc = rinp[:, 2 * ic + f:2 * ic + f + 1]
                        # r = relu(rin_f*S - 0.5); accum -> racc
                        rb = rbufp.tile([128, 1024], bf16, tag="rb",
                                        name=f"rb_{ic}_{jq}_{f}")
                        pi = (ic * JQ + jq) * 2 + f
                        nc.scalar.activation(rb, ps, AF.Relu, bias=negmargin,
                                             scale=rsc,
                                             accum_out=racc[:, pi:pi + 1])
                        rbufs.append(rb)
                        # accum L * S_raw -> pacc  (scaled by rin_f at the end)
                        scr = scrp.tile([128, 1024], bf16, tag="scr",
                                        name=f"scr_{ic}_{jq}_{f}")
                        qi = (2 * ic + f) * JQ + jq
                        nc.vector.scalar_tensor_tensor(
                            out=scr, in0=ps, scalar=1.0, in1=Lt[:, jsl],
                            op0=OP.bypass, op1=OP.mult,
                            accum_out=pacc[:, qi:qi + 1])
                    # w = r0 + r1 (bf16 2x); accum (w-2)*L -> qacc
                    w = rbufp.tile([128, 1024], bf16, tag="w",
                                   name=f"w_{ic}_{jq}")
                    nc.vector.tensor_add(w, rbufs[0], rbufs[1])
                    scr2 = scrp.tile([128, 1024], bf16, tag="scr2",
                                     name=f"scr2_{ic}_{jq}")
                    wi = ic * JQ + jq
                    nc.vector.scalar_tensor_tensor(
                        out=scr2, in0=w, scalar=2.0, in1=Lbs[ic][:, jsl],
                        op0=OP.subtract, op1=OP.mult,
                        accum_out=qacc[:, wi:wi + 1])

            # ---- finisher: out = sum(r) - sum((w-2)L) - sum(rin_f * pacc) ----
            pred = accp.tile([128, 2 * IT], f32)
            for g in range(2 * IT):
                nc.vector.reduce_sum(pred[:, g:g + 1],
                                     pacc[:, g * JQ:(g + 1) * JQ], axis=AX.X)
            scaled = accp.tile([128, 2 * IT], f32)
            nc.vector.tensor_mul(scaled, pred, rinp)
            ps_tot = small.tile([128, 1], f32, tag="fin", name="ps_tot")
            nc.vector.reduce_sum(ps_tot, scaled, axis=AX.X)
            ar = small.tile([128, 1], f32, tag="fin", name="ar")
            nc.vector.reduce_sum(ar, racc, axis=AX.X)
            qr = small.tile([128, 1], f32, tag="fin", name="qr")
            nc.vector.reduce_sum(qr, qacc, axis=AX.X)
            tmp = small.tile([128, 1], f32, tag="fin", name="tmp")
            nc.vector.tensor_sub(tmp, ar, qr)
            ov = small.tile([128, 1], f32, tag="fin", name="ov")
            nc.vector.tensor_sub(ov, tmp, ps_tot)
            nc.sync.dma_start(out_d, ov)

    nc.compile()
    return nc


def _get_nc():
    if "nc" not in _CACHE:
        _CACHE["nc"] = _build_nc()
    return _CACHE["nc"]


def _make_in_maps(fc_feats_0, fc_feats_1, textual_features, labels):
    in_maps = []
    for c in range(NCORES):
        sl = slice(c * ROWS, (c + 1) * ROWS)
        in_maps.append({
            "f0": np.ascontiguousarray(fc_feats_0[sl], dtype=np.float32),
            "f1": np.ascontiguousarray(fc_feats_1[sl], dtype=np.float32),
            "tx": np.ascontiguousarray(textual_features[sl], dtype=np.float32),
            "lab": np.ascontiguousarray(labels[sl], dtype=np.float32),
        })
    return in_maps


def run(fc_feats_0, fc_feats_1, textual_features, labels, trace=False):
    """Run on 8 NeuronCores; returns (loss_scalar, BassKernelResults)."""
    _import_concourse()
    from concourse.bass_utils import run_bass_kernel_spmd

    nc = _get_nc()
    in_maps = _make_in_maps(np.asarray(fc_feats_0), np.asarray(fc_feats_1),
                            np.asarray(textual_features), np.asarray(labels))
    res = run_bass_kernel_spmd(nc, in_maps, list(range(NCORES)), trace=trace)
    total = 0.0
    for c in range(NCORES):
        total += float(np.asarray(res.results[c]["outv"], dtype=np.float64).sum())
    loss = total / float(B * B)
    return np.asarray(loss, dtype=np.float32), res


def kernel(fc_feats_0, fc_feats_1, textual_features, labels):
    loss, _ = run(fc_feats_0, fc_feats_1, textual_features, labels, trace=False)
    return loss


# revision 11
# speedup vs baseline: 1.4195x; 1.4195x over previous
"""Trainium2 Bass kernel for nn_ContrastiveLoss (B=4096, D=1024, 8 cores).

loss = mean over [B,B] of
    labels*(1-sim0) + (1-labels)*relu(sim0-0.5)
  + labels*(1-sim1) + (1-labels)*relu(sim1-0.5)
where sim_k = cos_sim(fc_feats_k[i], textual_features[j]).

Strategy (data-parallel over rows):
  * Each of the 8 cores gets a 512-row slice of fc_feats_0/1 and labels.
  * textual_features row-shard is normalized+transposed to fp8 and
    AllGathered so every core holds the full tn^T.
  * f0/f1 are NOT normalized: raw rows are cast to fp8 and transposed;
    1/||f_i|| is applied as a per-partition ACT scale in the relu pass
    and factored out of the bilinear sums algebraically.
  * Per S-tile [128,1024] (2 j-chunks) in PSUM:
      ACT:  r = relu(rin_f * S - 0.5), accum -> racc   (A term + r data)
      DVE:  accum L * S_raw            -> pacc         (x rin_f later)
      DVE:  w = r0 + r1 (bf16, 2x rate)
      Pool: accum (w - 2) * L          -> qacc         (folds C and 2*sum(L))
  * total_core = sum_i [ A - q - rin0*p0 - rin1*p1 ]; host sums cores / B^2.

Self-contained: hardcodes shapes; only needs the concourse package.
"""

import os
import sys

import numpy as np

B = 4096
D = 1024
NCORES = 8
ROWS = B // NCORES          # 512 rows of f0/f1/labels per core
IT = ROWS // 128            # 4 i-tiles per core
KS = D // 128               # 8 k-subtiles (contraction)
JC = B // 512               # 8 j-chunks of 512 columns
JQ = 4                      # j "quarter" = 2 j-chunks = 1024 cols
MARGIN = 0.5
EPS = 1e-8
TN_SCALE = 64.0             # fp8 scale on normalized t rows

_CACHE = {}

# 2 = split the tn^T AllGather into two k-half collectives (pipelining);
# 1 = single collective.
N_COLL = int(os.environ.get("KERNEL_N_COLL", "1"))


def _import_concourse():
    try:
        import concourse.bass  # noqa: F401
    except ImportError:
        for p in ("/opt/trn_rl_repo", "/root/.axon_site/_ro/trn_rl_repo"):
            if os.path.isdir(p) and p not in sys.path:
                sys.path.insert(0, p)
        import concourse.bass  # noqa: F401


def _build_nc():
    """Build + schedule + compile the per-core Bass program (SPMD: same
    program on all 8 cores, different input slices)."""
    _import_concourse()
    import concourse.bass as bass  # noqa: F401
    import concourse.mybir as mybir
    import concourse.tile as tile
    from concourse import bacc
    from concourse.masks import make_identity

    f32 = mybir.dt.float32
    bf16 = mybir.dt.bfloat16
    fp8 = mybir.dt.float8e4
    AF = mybir.ActivationFunctionType
    OP = mybir.AluOpType
    AX = mybir.AxisListType
    DR = mybir.MatmulPerfMode.DoubleRow

    nc = bacc.Bacc(
        "TRN2",
        target_bir_lowering=False,
        debug=False,
        num_devices=NCORES,
    )

    f0_d = nc.dram_tensor("f0", [ROWS, D], f32, kind="ExternalInput").ap()
    f1_d = nc.dram_tensor("f1", [ROWS, D], f32, kind="ExternalInput").ap()
    tx_d = nc.dram_tensor("tx", [ROWS, D], f32, kind="ExternalInput").ap()
    lab_d = nc.dram_tensor("lab", [ROWS, B], f32, kind="ExternalInput").ap()
    out_d = nc.dram_tensor("outv", [128, 1], f32, kind="ExternalOutput").ap()

    with tile.TileContext(nc) as tc:
        with (
            tc.tile_pool(name="constp", bufs=1) as constp,
            tc.tile_pool(name="stage", bufs=4) as stage,
            tc.tile_pool(name="natbp", bufs=3) as natbp,
            tc.tile_pool(name="sqp", bufs=2) as sqp,
            tc.tile_pool(name="small", bufs=8) as small,
            tc.tile_pool(name="wT", bufs=1) as wTp,
            tc.tile_pool(name="tnTp", bufs=1) as tnTp,
            tc.tile_pool(name="labp", bufs=4) as labp,
            tc.tile_pool(name="rbufp", bufs=4) as rbufp,
            tc.tile_pool(name="scrp", bufs=4) as scrp,
            tc.tile_pool(name="accp", bufs=1) as accp,
            tc.tile_pool(name="tpsum", bufs=2, space="PSUM") as tpsum,
            tc.tile_pool(name="mpsum", bufs=3, space="PSUM") as mpsum,
            tc.tile_pool(name="dram", bufs=1, space="DRAM") as dram,
        ):
            ident = constp.tile([128, 128], bf16)
            make_identity(nc, ident)
            negmargin = constp.tile([128, 1], f32)
            nc.gpsimd.memset(negmargin, -MARGIN)

            if os.environ.get("KERNEL_WARMUP_COLL", "1") == "1":
                # Tiny AllGather issued first: absorbs the CC-engine warmup
                # latency so the real gather starts promptly.
                wsrc = dram.tile([1, 4], f32, name="warm_src")
                wdst = dram.tile([NCORES, 1, 4], f32, addr_space="Shared",
                                 name="warm_dst")
                wloc = constp.tile([1, 4], f32)
                nc.gpsimd.memset(wloc, 1.0)
                nc.gpsimd.dma_start(wsrc, wloc)
                nc.gpsimd.collective_compute(
                    "AllGather",
                    mybir.AluOpType.bypass,
                    replica_groups=[list(range(NCORES))],
                    ins=[wsrc.opt()],
                    outs=[wdst.opt()],
                )

            # persistent per-(ic,f) inverse norms (col 2*ic+f), scaled 1/64
            rinp = accp.tile([128, 2 * IT], f32)
            # accumulators, each column written exactly once
            racc = accp.tile([128, 2 * IT * JQ], f32)   # sum relu, (ic,jq,f)
            pacc = accp.tile([128, 2 * IT * JQ], f32)   # sum L*Sraw, (ic,f,jq)
            qacc = accp.tile([128, IT * JQ], f32)       # sum (w-2)*L, (ic,jq)

            # evacuation-copy engine rotation
            _evac_cnt = [0]

            def evac(dst, src):
                # Pool cannot read PSUM; alternate DVE / ACT only.
                e = _evac_cnt[0] % 2
                _evac_cnt[0] += 1
                if e == 0:
                    nc.vector.tensor_copy(dst, src)
                else:
                    nc.scalar.copy(dst, src)

            def norm_chain(nat, tag, scale_mul):
                """Square+accum -> sqrt -> clamp -> reciprocal -> *scale_mul.
                Returns the [128,1] f32 inverse-norm tile (not yet stored)."""
                sqb = sqp.tile([128, D], bf16, tag="sqb", name=f"sqb_{tag}")
                ssq = small.tile([128, 1], f32, tag="ssq", name=f"ssq_{tag}")
                nc.scalar.activation(sqb, nat, AF.Square, accum_out=ssq)
                nrm = small.tile([128, 1], f32, tag="nrm", name=f"nrm_{tag}")
                nc.scalar.activation(nrm, ssq, AF.Sqrt)
                nc.vector.tensor_scalar_max(nrm, nrm, EPS)
                rin = small.tile([128, 1], f32, tag="rin", name=f"rin_{tag}")
                nc.vector.reciprocal(rin, nrm)
                if scale_mul != 1.0:
                    nc.vector.tensor_scalar_mul(rin, rin, scale_mul)
                return rin

            def transpose_into(src_b, dst_T, it):
                """PE-transpose [128, D] bf16 tile into dst_T[:, ks, it*128:...]
                (fp8, layout [d_part, ks, row])."""
                for ks in range(KS):
                    pst = tpsum.tile([128, 128], bf16, tag="pst",
                                     name=f"pst_{dst_T.tensor.name}_{it}_{ks}")
                    nc.tensor.transpose(pst, src_b[:, ks * 128:(ks + 1) * 128],
                                        ident)
                    evac(dst_T[:, ks, it * 128:(it + 1) * 128], pst)

            # ---- phase A1: tx slice -> normalize -> transpose -> gather ----
            tT_loc = wTp.tile([128, KS, ROWS], fp8)
            for it in range(IT):
                nat = stage.tile([128, D], f32, tag="nat", name=f"tnat_{it}")
                nc.sync.dma_start(nat, tx_d[it * 128:(it + 1) * 128, :])
                rin = norm_chain(nat, f"t{it}", TN_SCALE)
                tnb = natbp.tile([128, D], bf16, tag="natb", name=f"tnb_{it}")
                nc.vector.tensor_scalar_mul(tnb, nat, rin)
                transpose_into(tnb, tT_loc, it)

            # share tn^T via AllGather (1 or 2 collectives of k-halves)
            kh = KS // N_COLL
            tT_loc_ds, tT_all_ds = [], []
            for h in range(N_COLL):
                tld = dram.tile([kh, 128, ROWS], fp8, name=f"tT_loc_d{h}")
                nc.gpsimd.dma_start(
                    tld.rearrange("ks p j -> p ks j"),
                    tT_loc[:, h * kh:(h + 1) * kh, :])
                tad = dram.tile([NCORES, kh, 128, ROWS], fp8,
                                addr_space="Shared", name=f"tT_all_d{h}")
                nc.gpsimd.collective_compute(
                    "AllGather",
                    mybir.AluOpType.bypass,
                    replica_groups=[list(range(NCORES))],
                    ins=[tld.opt()],
                    outs=[tad.opt()],
                )
                tT_loc_ds.append(tld)
                tT_all_ds.append(tad)

            # ---- phase A2: f0/f1 raw -> cast -> norms -> transpose ----
            f0T = wTp.tile([128, KS, ROWS], fp8)
            f1T = wTp.tile([128, KS, ROWS], fp8)
            for f, (src_ap, fT) in enumerate(((f0_d, f0T), (f1_d, f1T))):
                for it in range(IT):
                    nat = stage.tile([128, D], f32, tag="nat",
                                     name=f"fnat_{f}_{it}")
                    nc.sync.dma_start(nat, src_ap[it * 128:(it + 1) * 128, :])
                    natb = natbp.tile([128, D], bf16, tag="natb",
                                      name=f"fnatb_{f}_{it}")
                    nc.vector.tensor_copy(natb, nat)
                    rin = norm_chain(nat, f"f{f}_{it}", 1.0 / TN_SCALE)
                    nc.vector.tensor_copy(rinp[:, 2 * it + f:2 * it + f + 1],
                                          rin)
                    transpose_into(natb, fT, it)

            # ---- labels: fully resident, loaded early ----
            Lts = []
            for ic in range(IT):
                Lt = labp.tile([128, B], f32, tag="Lt", name=f"Lt_{ic}")
                nc.sync.dma_start(Lt, lab_d[ic * 128:(ic + 1) * 128, :])
                Lts.append(Lt)

            # ---- gathered tn^T -> SBUF [128, jc*KS + ks, 512] ----
            tnT = tnTp.tile([128, JC * KS, 512], fp8)
            for jc in range(JC):
                for h in range(N_COLL):
                    nc.sync.dma_start(
                        tnT[:, jc * KS + h * kh: jc * KS + (h + 1) * kh, :],
                        tT_all_ds[h][jc].rearrange("ks p j -> p ks j"),
                    )

            # ---- phase B: matmuls + fused loss ----
            for ic in range(IT):
                isl = slice(ic * 128, (ic + 1) * 128)
                Lt = Lts[ic]
                for jq in range(JQ):
                    jsl = slice(jq * 1024, (jq + 1) * 1024)
                    rbufs = []
                    for f, fT in enumerate((f0T, f1T)):
                        ps = mpsum.tile([128, 1024], f32, tag="ps",
                                        name=f"ps_{ic}_{jq}_{f}")
                        for jh in range(2):
                            jc = jq * 2 + jh
                            for k2 in range(KS // 2):
                                ksl = slice(jc * KS + 2 * k2,
                                            jc * KS + 2 * k2 + 2)
                                nc.tensor.matmul(
                                    ps[:, jh * 512:(jh + 1) * 512],
                                    fT[:, 2 * k2:2 * k2 + 2, isl],
                                    tnT[:, ksl, :], perf_mode=DR,
                                    start=(k2 == 0), stop=(k2 == KS // 2 - 1),
                                )
                        rsc = rinp[:, 2 * ic + f:2 * ic + f + 1]
                        # r = relu(rin_f*S - 0.5); accum -> racc
                        rb = rbufp.tile([128, 1024], bf16, tag="rb",
                                        name=f"rb_{ic}_{jq}_{f}")
                        pi = (ic * JQ + jq) * 2 + f
                        nc.scalar.activation(rb, ps, AF.Relu, bias=negmargin,
                                             scale=rsc,
                                             accum_out=racc[:, pi:pi + 1])
                        rbufs.append(rb)
                        # accum L * S_raw -> pacc  (scaled by rin_f at the end)
                        scr = scrp.tile([128, 1024], bf16, tag="scr",
                                        name=f"scr_{ic}_{jq}_{f}")
                        qi = (2 * ic + f) * JQ + jq
                        nc.vector.scalar_tensor_tensor(
                            out=scr, in0=ps, scalar=1.0, in1=Lt[:, jsl],
                            op0=OP.bypass, op1=OP.mult,
                            accum_out=pacc[:, qi:qi + 1])
                    # w = r0 + r1 (bf16 2x); accum (w-2)*L -> qacc
                    w = rbufp.tile([128, 1024], bf16, tag="w",
                                   name=f"w_{ic}_{jq}")
                    nc.vector.tensor_add(w, rbufs[0], rbufs[1])
                    scr2 = scrp.tile([128, 1024], bf16, tag="scr2",
                                     name=f"scr2_{ic}_{jq}")
                    wi = ic * JQ + jq
                    nc.vector.scalar_tensor_tensor(
                        out=scr2, in0=w, scalar=2.0, in1=Lt[:, jsl],
                        op0=OP.subtract, op1=OP.mult,
                        accum_out=qacc[:, wi:wi + 1])

            # ---- finisher: out = sum(r) - sum((w-2)L) - sum(rin_f * pacc) ----
            pred = accp.tile([128, 2 * IT], f32)
            for g in range(2 * IT):
                nc.vector.reduce_sum(pred[:, g:g + 1],
                                     pacc[:, g * JQ:(g + 1) * JQ], axis=AX.X)
            scaled = accp.tile([128, 2 * IT], f32)
            nc.vector.tensor_mul(scaled, pred, rinp)
            ps_tot = small.tile([128, 1], f32, tag="fin", name="ps_tot")
            nc.vector.reduce_sum(ps_tot, scaled, axis=AX.X)
            ar = small.tile([128, 1], f32, tag="fin", name="ar")
            nc.vector.reduce_sum(ar, racc, axis=AX.X)
            qr = small.tile([128, 1], f32, tag="fin", name="qr")
            nc.vector.reduce_sum(qr, qacc, axis=AX.X)
            tmp = small.tile([128, 1], f32, tag="fin", name="tmp")
            nc.vector.tensor_sub(tmp, ar, qr)
            ov = small.tile([128, 1], f32, tag="fin", name="ov")
            nc.vector.tensor_sub(ov, tmp, ps_tot)
            nc.sync.dma_start(out_d, ov)

    nc.compile()
    return nc


def _get_nc():
    if "nc" not in _CACHE:
        _CACHE["nc"] = _build_nc()
    return _CACHE["nc"]


def _make_in_maps(fc_feats_0, fc_feats_1, textual_features, labels):
    in_maps = []
    for c in range(NCORES):
        sl = slice(c * ROWS, (c + 1) * ROWS)
        in_maps.append({
            "f0": np.ascontiguousarray(fc_feats_0[sl], dtype=np.float32),
            "f1": np.ascontiguousarray(fc_feats_1[sl], dtype=np.float32),
            "tx": np.ascontiguousarray(textual_features[sl], dtype=np.float32),
            "lab": np.ascontiguousarray(labels[sl], dtype=np.float32),
        })
    return in_maps


def run(fc_feats_0, fc_feats_1, textual_features, labels, trace=False):
    """Run on 8 NeuronCores; returns (loss_scalar, BassKernelResults)."""
    _import_concourse()
    from concourse.bass_utils import run_bass_kernel_spmd

    nc = _get_nc()
    in_maps = _make_in_maps(np.asarray(fc_feats_0), np.asarray(fc_feats_1),
                            np.asarray(textual_features), np.asarray(labels))
    res = run_bass_kernel_spmd(nc, in_maps, list(range(NCORES)), trace=trace)
    total = 0.0
    for c in range(NCORES):
        total += float(np.asarray(res.results[c]["outv"], dtype=np.float64).sum())
    loss = total / float(B * B)
    return np.asarray(loss, dtype=np.float32), res


def kernel(fc_feats_0, fc_feats_1, textual_features, labels):
    loss, _ = run(fc_feats_0, fc_feats_1, textual_features, labels, trace=False)
    return loss
